# revision 13
# baseline (speedup 1.0000x reference)
import sys

sys.path.insert(0, "/opt/trn_rl_repo")

import numpy as np
from contextlib import ExitStack
from concourse import bacc, bass_utils, tile, mybir

F32 = mybir.dt.float32
F16 = mybir.dt.float16
TANH = mybir.ActivationFunctionType.Tanh

NCORES = 8


def _build(nu, d_idx, steps, ncores):
    # Per-core layout: N = 16384 samples, 2 per column -> 8192 sample
    # cols in 16 chunks of 512. PSUM: a_ring [128,3584] (7 banks) holds
    # pre-activations in a 7-slot ring (global chunk g -> slot g%7);
    # y_bank [128,512] (1 bank) holds the persistent fp32 y state:
    # chunk (j=strip, b) -> rows 32j+4b+{2AB+f}. mm2 writes a full
    # 32-row strip with a zero-padded lhsT so sibling chunks' y rows
    # get +0 (exact, preserved) and its own 4 rows accumulate
    # dt*(W2 h + b2). tanh runs on big groups of 4/3 chunks
    # ([100,2048]/[100,1536]) instead of per-chunk, and the y recirc
    # is two [64,512] DVE copies per step (halved by strip pairs so
    # next-step mm1s of half H wait only on copy-H).
    nc = bacc.Bacc(
        "TRN2",
        target_bir_lowering=False,
        debug=False,
        enable_asserts=False,
        num_devices=ncores,
    )
    w1z_d = nc.dram_tensor("w1z", [128, 400], F16, kind="ExternalInput")
    w2z_d = nc.dram_tensor("w2z", [101, 128 * nu], F16, kind="ExternalInput")
    b1bd_d = nc.dram_tensor("b1bd", [100, 1], F32, kind="ExternalInput")
    y0f_d = nc.dram_tensor("y0f", [128, 512], F32, kind="ExternalInput")
    eye_d = nc.dram_tensor("eye32", [128, 32], F32, kind="ExternalInput")
    out_d = nc.dram_tensor("out", [steps, 128, 512], F16, kind="ExternalOutput")

    CH = steps * 16
    with tile.TileContext(nc) as tc:
        with ExitStack() as ctx:
            sb = ctx.enter_context(tc.tile_pool(name="sb", bufs=1, space="SBUF"))
            ps = ctx.enter_context(tc.tile_pool(name="ps", bufs=1, space="PSUM"))

            w1z = sb.tile([128, 400], F16, tag="w1", name="w1z")
            w2z = sb.tile([101, 128 * nu], F16, tag="w2", name="w2z")
            b1bd = sb.tile([100, 1], F32, tag="b1", name="b1bd")
            y0f = sb.tile([128, 512], F32, tag="y0", name="y0f")
            eye = sb.tile([128, 32], F32, tag="ey", name="eye32")
            yr = sb.tile([128, 512], F16, tag="yr", name="yr")
            hbufs = [
                sb.tile([101, 2048 if i % 2 == 0 else 1536], F16,
                        tag=f"h{i}", name=f"h{i}")
                for i in range(4)
            ]
            ones = sb.tile([101, 2048], F32, tag="on", name="ones")

            a_ring = ps.tile([128, 3584], F32, tag="ar", name="a_ring")
            y_bank = ps.tile([128, 512], F32, tag="yb", name="y_bank")

            nc.sync.dma_start(out=w1z[:, :], in_=w1z_d[:, :])
            nc.sync.dma_start(out=w2z[:, :], in_=w2z_d[:, :])
            nc.sync.dma_start(out=b1bd[:, :], in_=b1bd_d[:, :])
            nc.sync.dma_start(out=y0f[:, :], in_=y0f_d[:, :])
            nc.sync.dma_start(out=eye[:, :], in_=eye_d[:, :])

            # h row 100 is the constant-1 bias row (mm2 adds dt*b2 via
            # lhsT row 100). memset can't write F16 reliably and engine
            # partition bases must be 32-aligned, so stage rows 96-100
            # in F32 and copy; rows 96-99 get overwritten by every tanh.
            nc.vector.memset(ones[96:101, :], 1.0)
            for i in range(4):
                w = 2048 if i % 2 == 0 else 1536
                nc.vector.tensor_copy(
                    out=hbufs[i][96:101, :], in_=ones[96:101, 0:w]
                )

            # init: y_bank = y0 (exact fp32) via identity matmul
            for j in range(4):
                nc.tensor.matmul(
                    y_bank[32 * j : 32 * j + 32, :],
                    lhsT=eye[32 * j : 32 * j + 32, :],
                    rhs=y0f[32 * j : 32 * j + 32, :],
                    start=True,
                    stop=True,
                    tile_position=(32 * j, 32 * j),
                )
            nc.vector.tensor_copy(out=yr[0:64, :], in_=y_bank[0:64, :])
            nc.vector.tensor_copy(out=yr[64:128, :], in_=y_bank[64:128, :])

            def chunk_geo(g):
                s, c = divmod(g, 16)
                H, idx = divmod(c, 8)
                j = 2 * H + idx % 2
                b = idx // 2
                return s, c, j, b

            def mm1(g):
                s, c, j, b = chunk_geo(g)
                slot = g % 7
                nc.tensor.matmul(
                    a_ring[0:100, 512 * slot : 512 * slot + 512],
                    lhsT=w1z[32 * j : 32 * j + 32, 100 * b : 100 * b + 100],
                    rhs=yr[32 * j : 32 * j + 32, :],
                    start=True,
                    stop=True,
                    tile_position=(32 * j, 0),
                )

            def tanh(k, gs):
                buf = hbufs[k % 4]
                slot0 = 0 if k % 2 == 0 else 4
                nc.scalar.activation(
                    buf[0:100, 0 : 512 * gs],
                    a_ring[0:100, 512 * slot0 : 512 * slot0 + 512 * gs],
                    TANH,
                    bias=b1bd[:, :],
                )

            def mm2(g, k, pos):
                s, c, j, b = chunk_geo(g)
                d = d_idx[s]
                buf = hbufs[k % 4]
                nc.tensor.matmul(
                    y_bank[32 * j : 32 * j + 32, :],
                    lhsT=w2z[:, 32 * (4 * d + b) : 32 * (4 * d + b) + 32],
                    rhs=buf[0:101, 512 * pos : 512 * pos + 512],
                    start=False,
                    stop=True,
                    tile_position=(0, 32 * j),
                    skip_group_check=True,
                )

            def group_chunks(k):
                p, e = divmod(k, 2)
                g0 = 7 * p + (0 if e == 0 else 4)
                gs = 4 if e == 0 else 3
                return [g for g in range(g0, g0 + gs) if g < CH]

            # Emit mm1s one group ahead of mm2s so the PE can prefetch
            # the next group's pre-activations while ACT runs tanh and
            # before the h-gated mm2s; 4 h buffers keep tanh(k) clear
            # of mm2(k-2)'s reads.
            K = 2 * ((CH + 6) // 7)
            for k in range(K + 1):
                cur = group_chunks(k) if k < K else []
                if k == 0:
                    for g in cur:
                        mm1(g)
                if k + 1 < K:
                    for g in group_chunks(k + 1):
                        mm1(g)
                if k >= 1:
                    prev = group_chunks(k - 1)
                    for pos, g in enumerate(prev):
                        mm2(g, k - 1, pos)
                    for g in prev:
                        s, c = divmod(g, 16)
                        if c == 7:
                            nc.vector.tensor_copy(
                                out=yr[0:64, :], in_=y_bank[0:64, :]
                            )
                            nc.sync.dma_start(
                                out=out_d[s, 0:64, :], in_=yr[0:64, :]
                            )
                        elif c == 15:
                            nc.vector.tensor_copy(
                                out=yr[64:128, :], in_=y_bank[64:128, :]
                            )
                            nc.sync.dma_start(
                                out=out_d[s, 64:128, :], in_=yr[64:128, :]
                            )
                if cur:
                    tanh(k, len(cur))
    nc.compile()
    return nc


def _prep(y0, t, w1, b1, w2, b2, ncores):
    B = y0.shape[0]
    steps = t.shape[0] - 1
    N = B // ncores
    dts = (t[1:] - t[:-1]).astype(np.float32)
    uniq, inv = np.unique(dts, return_inverse=True)
    nu = len(uniq)

    w1 = np.asarray(w1, np.float64)
    w2 = np.asarray(w2, np.float64)
    b1 = np.asarray(b1, np.float64)
    b2 = np.asarray(b2, np.float64)

    # mm1 lhsT, 4 b-variants: rows 4b+2AB+f select yr rows 32j+4b+..,
    # cols 50AB+u produce a[50AB+u] = sum_f w1[u,f] y[AB,f]; replicated
    # at all 4 strips (lhsT must share rhs's partition base).
    w1z = np.zeros((32, 400), np.float64)
    for b in range(4):
        for AB in range(2):
            for f in range(2):
                w1z[4 * b + 2 * AB + f, 100 * b + 50 * AB : 100 * b + 50 * AB + 50] = w1[:, f]
    w1z = np.tile(w1z.astype(np.float16), (4, 1))

    # mm2 lhsT, (d, b)-variants [101, 32]: col q=4b+2AB+f gets
    # dt_d*w2[f,:] against h rows 50AB+u, plus dt_d*b2[f] at ones row
    # 100; all other cols zero so sibling y rows accumulate +0.
    w2z = np.zeros((101, 128 * nu), np.float64)
    for d in range(nu):
        dt = float(uniq[d])
        for b in range(4):
            o = 32 * (4 * d + b)
            for AB in range(2):
                for f in range(2):
                    w2z[50 * AB : 50 * AB + 50, o + 4 * b + 2 * AB + f] = dt * w2[f, :]
                    w2z[100, o + 4 * b + 2 * AB + f] = dt * b2[f]
    w2z = w2z.astype(np.float16)

    b1bd = np.concatenate([b1, b1]).astype(np.float32).reshape(100, 1)
    eye32 = np.tile(np.eye(32, dtype=np.float32), (4, 1))

    # y row = 64H + 32jj + 4b + 2AB + f <- sample 8192H+2048b+1024jj+512AB+i
    in_maps = []
    for k in range(ncores):
        blk = np.asarray(y0[k * N : (k + 1) * N], np.float32).reshape(
            2, 4, 2, 2, 512, 2
        )  # [H, b, jj, AB, i, f]
        tmp = blk.transpose(0, 2, 1, 3, 5, 4)  # [H, jj, b, AB, f, i]
        y0f = np.zeros((2, 2, 2, 4, 2, 2, 512), np.float32)  # [H,jj,up,b,AB,f,i]
        y0f[:, :, 0] = tmp
        in_maps.append(
            {
                "w1z": w1z,
                "w2z": w2z,
                "b1bd": b1bd,
                "y0f": y0f.reshape(128, 512),
                "eye32": eye32,
            }
        )
    return nu, list(inv), steps, N, in_maps


def run(y0, t, w1, b1, w2, b2, ncores=NCORES, steps_override=None, trace=False):
    y0 = np.ascontiguousarray(y0, dtype=np.float32)
    nu, inv, steps, N, in_maps = _prep(
        y0, np.asarray(t), np.asarray(w1), np.asarray(b1), np.asarray(w2),
        np.asarray(b2), ncores,
    )
    if steps_override is not None:
        steps = steps_override
    nc = _build(nu, inv, steps, ncores)
    res = bass_utils.run_bass_kernel_spmd(
        nc, in_maps, list(range(ncores)), trace=trace
    )
    B = y0.shape[0]
    out = np.empty((steps + 1, B, 2), np.float32)
    out[0] = y0
    for k in range(ncores):
        v = np.asarray(res.results[k]["out"]).astype(np.float32)
        v = v.reshape(steps, 2, 2, 2, 4, 2, 2, 512)[:, :, :, 0]
        # [s, H, jj, b, AB, f, i] -> [s, H, b, jj, AB, i, f]
        out[1:, k * N : (k + 1) * N, :] = (
            v.transpose(0, 1, 3, 2, 4, 6, 5).reshape(steps, N, 2)
        )
    return out, res


def kernel(**inputs):
    out, _ = run(
        inputs["y0"], inputs["t"], inputs["w1"], inputs["b1"], inputs["w2"],
        inputs["b2"],
    )
    return out
